# revision 1
# baseline (speedup 1.0000x reference)
"""MultiHeadAttention Trainium2 Bass kernel.

Problem: B=2, S=2048, D=768, H=12 heads, head_dim=64.
    q = x@Wq+bq; k = x@Wk+bk; v = x@Wv+bv   (per-head split)
    out = softmax(q k^T / 8) v, heads merged, @ Wo + bo

Sharding (8 cores): core c handles batch b=c//4 and 3 heads (c%4)*3..+3
(Megatron attention: column-split of Wq/Wk/Wv, row-split of Wo). Each core
produces a partial [S, D] output; the host sums the 4 partials per batch and
adds (bv @ Wo + bo) once (the bv contribution passes through softmax rows
that sum to 1, so it is folded on the host).

Per-core device kernel (fp32 data, float32r matmuls = 4x PE throughput):
  - loads xT = x[b]^T as [128, 6, 2048] (contraction dim on partitions)
  - qT/kT = W^T @ xT via PE, bias added per-partition on eviction
    (Wq and bq pre-scaled by 1/8 on host so scores = qT^T kT needs no scale)
  - v = x @ Wv per 128-row tile, stored with a ones-column per head
  - attention processes heads 0+1 as a pair (packed side by side in one
    [128, 1024] scores tile; the two matmuls use disjoint PE row groups so
    they overlap), head 2 alone, per 16 key-tiles j:
      scoresT[j] = k^T-block @ qT    -> PSUM
      expT = Exp(scoresT) on ScalarE (no max-subtraction: scores ~N(0,1))
      ctxT[65, 1024] += [v | 1]^T @ expT   (row 64 = softmax denominator)
    then ctxT normalized: 1/denom via DVE reciprocal_approx_fast, broadcast
    across partitions on GpSimd, multiplied on DVE during PSUM eviction
  - out_partial = ctxT^T @ Wo_slice per 128-row tile -> DMA to HBM
  - emission order interleaves attention with projection chunks so the
    ScalarE exp stream (the second-busiest engine) starts early

kernel(**inputs) takes FULL unsharded inputs and returns the FULL output.
"""

import numpy as np

import concourse.bass as bass
import concourse.mybir as mybir
import concourse.tile as tile
from concourse import bacc
from concourse.bass_utils import run_bass_kernel_spmd

F32 = mybir.dt.float32
F32R = mybir.dt.float32r  # fp32 data, reduced-precision matmul (1 cyc/row vs 4)

B, S, D = 2, 2048, 768
H, DH = 12, 64
NCORES = 8
HPC = 3                # heads per core
DH3 = HPC * DH         # 192 (per-core slice of the model dim)
KT = D // 128          # 6 contraction tiles for D
ST = S // 128          # 16 sequence tiles
QC = 1024              # q-chunk width in the attention inner loop
NQC = S // QC          # 2
SC = 512               # matmul moving-operand max (fp32)

_CACHED_NC = None


def _build_nc(debug: bool = False) -> bass.Bass:
    nc = bacc.Bacc()

    xT = nc.dram_tensor("xT", [D, S], F32R, kind="ExternalInput")
    wq = nc.dram_tensor("wq", [D, 128], F32R, kind="ExternalInput")
    wk = nc.dram_tensor("wk", [D, 128], F32R, kind="ExternalInput")
    wkq2 = nc.dram_tensor("wkq2", [D, 128], F32R, kind="ExternalInput")
    wv = nc.dram_tensor("wv", [D, DH3 + 64], F32R, kind="ExternalInput")
    wo = nc.dram_tensor("wo", [DH3, D], F32R, kind="ExternalInput")
    bias = nc.dram_tensor("bias", [128, 4], F32, kind="ExternalInput")
    out = nc.dram_tensor("out", [S, D], F32, kind="ExternalOutput")
    if debug:
        d_qTA = nc.dram_tensor("d_qTA", [128, S], F32R, kind="ExternalOutput")
        d_qTB = nc.dram_tensor("d_qTB", [64, S], F32R, kind="ExternalOutput")
        d_kTA = nc.dram_tensor("d_kTA", [128, S], F32R, kind="ExternalOutput")
        d_kTB = nc.dram_tensor("d_kTB", [64, S], F32R, kind="ExternalOutput")
        d_v = nc.dram_tensor("d_v", [128, ST * HPC * (DH + 1)], F32R,
                             kind="ExternalOutput")
        d_ctxA = nc.dram_tensor("d_ctxA", [128, S], F32R, kind="ExternalOutput")
        d_ctxB = nc.dram_tensor("d_ctxB", [64, S], F32R, kind="ExternalOutput")

    with (
        tile.TileContext(nc) as tc,
        tc.tile_pool(name="big", bufs=1) as big,
        tc.tile_pool(name="work", bufs=2) as work,
        tc.tile_pool(name="expp", bufs=5) as expp,
        tc.tile_pool(name="psA", bufs=2, space="PSUM") as psA,
        tc.tile_pool(name="psB", bufs=2, space="PSUM") as psB,
    ):
        # ---- persistent SBUF tensors (f32r: rounded inputs for fast matmul) ----
        x_sb = big.tile([128, KT, S], F32R)         # xT: [p, ktile, s]
        wq_sb = big.tile([128, KT, 128], F32R)
        wk_sb = big.tile([128, KT, 128], F32R)
        wkq2_sb = big.tile([128, KT, 128], F32R)  # [k_h2 | q_h2] combined
        wv_sb = big.tile([128, KT, DH3 + 64], F32R)  # padded to N=256 for f32r
        woA_sb = big.tile([128, D], F32R)           # Wo rows 0..127
        woB_sb = big.tile([64, D], F32R)            # Wo rows 128..191
        bias_sb = big.tile([128, 4], F32)  # [bk01 | bk2@0:64 | bq01 | bq2@64:128]
        ones_col = big.tile([1, 64], F32)           # lhsT for denom broadcast
        qTA = big.tile([128, S], F32R)              # qT heads 0,1
        qTB = big.tile([128, S], F32R)              # qT head 2 in rows 64..127
        kTA = big.tile([128, S], F32R)
        kTB = big.tile([128, S], F32R)              # kT head 2 in rows 64..127
        ctxA = big.tile([128, S], F32R)             # normalized ctx^T heads 0,1
        ctxB = big.tile([64, S], F32R)              # head 2
        v_sb = big.tile([128, ST, HPC, DH + 1], F32R)  # v tiles + ones column

        # ---- loads ----
        # Order: first-needed first; x streamed in 4 column chunks so
        # projections start before the full 6MB lands.
        nc.scalar.dma_start(out=wk_sb, in_=wk.rearrange("(kt p) m -> p kt m", p=128))
        nc.sync.dma_start(
            out=x_sb[:, :, 0:SC],
            in_=xT[:, 0:SC].rearrange("(kt p) q -> p kt q", p=128),
        )
        nc.scalar.dma_start(out=wq_sb, in_=wq.rearrange("(kt p) m -> p kt m", p=128))
        nc.scalar.dma_start(out=bias_sb, in_=bias[:, :])
        nc.sync.dma_start(
            out=x_sb[:, :, SC : 2 * SC],
            in_=xT[:, SC : 2 * SC].rearrange("(kt p) q -> p kt q", p=128),
        )
        nc.scalar.dma_start(out=wkq2_sb, in_=wkq2.rearrange("(kt p) m -> p kt m", p=128))
        nc.scalar.dma_start(out=wv_sb, in_=wv.rearrange("(kt p) m -> p kt m", p=128))
        for c in range(2, 4):
            cs = slice(c * SC, (c + 1) * SC)
            nc.sync.dma_start(
                out=x_sb[:, :, cs],
                in_=xT[:, cs].rearrange("(kt p) q -> p kt q", p=128),
            )
        nc.scalar.dma_start(out=woA_sb, in_=wo[0:128, :])
        nc.scalar.dma_start(out=woB_sb, in_=wo[128:DH3, :])
        nc.vector.memset(ones_col, 1.0)
        nc.vector.memset(v_sb[:, :, :, DH : DH + 1].bitcast(F32), 1.0)

        # head h slices of the packed qT/kT/ctxT tiles
        def head_sl(tA, tB, h, fsl):
            if h == 0:
                return tA[0:64, fsl]
            if h == 1:
                return tA[64:128, fsl]
            return tB[0:64, fsl]

        # ---- stage helpers (emission order below sets scheduler priority) ----
        def _proj_mm(w_sb, c):
            cs = slice(c * SC, (c + 1) * SC)
            ps_qk = psA.tile([128, SC], F32, tag="a", name="ps_qk")
            for kt in range(KT):
                nc.tensor.matmul(
                    ps_qk,
                    lhsT=w_sb[:, kt, :],
                    rhs=x_sb[:, kt, cs],
                    start=(kt == 0),
                    stop=(kt == KT - 1),
                )
            return ps_qk, cs

        def proj_k(c):
            ps, cs = _proj_mm(wk_sb, c)
            nc.vector.tensor_scalar_add(kTA[:, cs], ps, bias_sb[:, 0:1])

        def proj_q(c):
            ps, cs = _proj_mm(wq_sb, c)
            nc.vector.tensor_scalar_add(qTA[:, cs], ps, bias_sb[:, 2:3])

        def proj_kq2(c):
            # combined head-2 projection: psum rows 0:64 = kT_h2, 64:128 =
            # qT_h2. k evicts with an up-shift to rows 64..127 of kTB; q
            # evicts in place (all APs at base 64) so the head-2 scores
            # matmul sees base-aligned operands.
            ps, cs = _proj_mm(wkq2_sb, c)
            nc.vector.tensor_scalar_add(kTB[64:128, cs], ps[0:64, :], bias_sb[0:64, 1:2])
            nc.vector.tensor_scalar_add(
                qTB[64:128, cs], ps[64:128, :], bias_sb[64:128, 3:4]
            )

        def proj_v_st(st):
            ss = slice(st * 128, (st + 1) * 128)
            ps_v = psA.tile([128, DH3 + 64], F32, tag="a", name="ps_v")
            for kt in range(KT):
                nc.tensor.matmul(
                    ps_v,
                    lhsT=x_sb[:, kt, ss],
                    rhs=wv_sb[:, kt, :],
                    start=(kt == 0),
                    stop=(kt == KT - 1),
                )
            nc.vector.tensor_copy(
                v_sb[:, st, :, 0:DH],
                ps_v[:, 0:DH3].rearrange("p (h d) -> p h d", h=HPC),
            )

        def proj_v(c):
            for st in range(4 * c, 4 * c + 4):
                proj_v_st(st)

        ctx_psums = {}  # group key -> ps_ctx tile, allocated on first j-part

        def _normalize(ps_ctx, dsts):
            # normalize: ctx^T[d, q] / denom[q]  (denom in psum row 64).
            # Custom-DVE ops mis-execute at partition base != 0, and PSUM
            # reads can't shift partitions down — stage the denom row through
            # SBUF@64 then SBUF@0 with plain copies, then broadcast 1/denom
            # across partitions on GpSimd.  dsts: list of (ctx_dst_ap, col0).
            den65 = work.tile([DH + 1, QC], F32, tag="den65", name="den65")
            nc.vector.tensor_copy(den65[DH : DH + 1, :], ps_ctx[DH : DH + 1, :])
            den0 = work.tile([1, QC], F32, tag="den0", name="den0")
            nc.vector.tensor_copy(den0, den65[DH : DH + 1, :])
            rden = work.tile([1, QC], F32, tag="rden", name="rden")
            nc.vector.reciprocal_approx_fast(out=rden, in_=den0)
            bc_sb = work.tile([64, QC], F32, tag="bc_sb", name="bc_sb")
            nc.gpsimd.partition_broadcast(bc_sb, rden)
            for dst, col0, csz in dsts:
                nc.vector.tensor_mul(
                    dst,
                    ps_ctx[0:DH, col0 : col0 + csz],
                    bc_sb[:, col0 : col0 + csz],
                )

        def pair_part(g, j0, j1):
            # heads 0+1 together: 512 q-columns each, packed side by side in
            # one [128, 1024] scores tile / one [65, 1024] ctx tile. The two
            # scores matmuls use disjoint PE row groups (kTA rows 0:64 vs
            # 64:128) and different PSUM banks, so they overlap on hardware.
            qs = slice(g * 512, (g + 1) * 512)
            key = ("pair", g)
            if key not in ctx_psums:
                ctx_psums[key] = psB.tile([DH + 1, QC], F32, tag="b", name="ps_ctx")
            ps_ctx = ctx_psums[key]
            for j in range(j0, j1):
                js = slice(j * 128, (j + 1) * 128)
                ps_sc = psA.tile([128, QC], F32, tag="a", name="ps_sc")
                nc.tensor.matmul(
                    ps_sc[:, 0:512], lhsT=kTA[0:64, js], rhs=qTA[0:64, qs],
                    start=True, stop=True,
                )
                nc.tensor.matmul(
                    ps_sc[:, 512:1024], lhsT=kTA[64:128, js], rhs=qTA[64:128, qs],
                    start=True, stop=True,
                )
                expT = expp.tile([128, QC], F32R, tag="expT", name="expT")
                nc.scalar.activation(expT, ps_sc, mybir.ActivationFunctionType.Exp)
                nc.tensor.matmul(
                    ps_ctx[:, 0:512], lhsT=v_sb[:, j, 0, :], rhs=expT[:, 0:512],
                    start=(j == 0), stop=(j == ST - 1),
                )
                nc.tensor.matmul(
                    ps_ctx[:, 512:1024], lhsT=v_sb[:, j, 1, :],
                    rhs=expT[:, 512:1024],
                    start=(j == 0), stop=(j == ST - 1),
                )
            if j1 < ST:
                return
            _normalize(
                ps_ctx,
                [(ctxA[0:64, qs], 0, 512), (ctxA[64:128, qs], 512, 512)],
            )
            del ctx_psums[key]

        def h2_part(q, j0, j1):
            # head 2 alone: full 1024-wide q-chunk as two 512 column halves
            key = ("h2", q)
            if key not in ctx_psums:
                ctx_psums[key] = psB.tile([DH + 1, QC], F32, tag="b", name="ps_ctx")
            ps_ctx = ctx_psums[key]
            for j in range(j0, j1):
                js = slice(j * 128, (j + 1) * 128)
                ps_sc = psA.tile([128, QC], F32, tag="a", name="ps_sc")
                for c2 in range(QC // SC):
                    qcs = slice(q * QC + c2 * SC, q * QC + (c2 + 1) * SC)
                    nc.tensor.matmul(
                        ps_sc[:, c2 * SC : (c2 + 1) * SC],
                        lhsT=kTB[64:128, js],
                        rhs=qTB[64:128, qcs],
                        start=True,
                        stop=True,
                    )
                expT = expp.tile([128, QC], F32R, tag="expT", name="expT")
                nc.scalar.activation(expT, ps_sc, mybir.ActivationFunctionType.Exp)
                for c2 in range(QC // SC):
                    c2s = slice(c2 * SC, (c2 + 1) * SC)
                    nc.tensor.matmul(
                        ps_ctx[:, c2s],
                        lhsT=v_sb[:, j, 2, :],
                        rhs=expT[:, c2s],
                        start=(j == 0),
                        stop=(j == ST - 1),
                    )
            if j1 < ST:
                return
            qf = slice(q * QC, (q + 1) * QC)
            _normalize(ps_ctx, [(ctxB[0:64, qf], 0, QC)])
            del ctx_psums[key]

        def out_proj(st):
            ss = slice(st * 128, (st + 1) * 128)
            ps_o = psA.tile([128, D], F32, tag="a", name="ps_o")
            # ctxB (head 2) first: its normalize finishes before the final
            # pair group's, so the tail out-projs can start earlier
            for c2, csz in ((0, 512), (1, 256)):
                osl = slice(c2 * 512, c2 * 512 + csz)
                nc.tensor.matmul(
                    ps_o[:, osl], lhsT=ctxB[:, ss], rhs=woB_sb[:, osl],
                    start=True, stop=False,
                )
                nc.tensor.matmul(
                    ps_o[:, osl], lhsT=ctxA[:, ss], rhs=woA_sb[:, osl],
                    start=False, stop=True,
                )
            o_sb = expp.tile([128, D], F32, tag="o_sb", name="o_sb")
            nc.vector.tensor_copy(o_sb, ps_o)
            nc.sync.dma_start(out=out[ss, :], in_=o_sb)

        # ---- emission order: attention parts interleave with projection
        # chunks so the ACT exp stream starts as early as possible and PE
        # always has filler; the Tile scheduler resolves the actual deps.
        # At most 2 ctx psum groups may be open at once (pool bufs=2).
        # pair group g needs qT chunk g; its j-tiles 4c..4c+3 need kT/v chunk c.
        for c in range(2):
            proj_k(c)
            proj_q(c)
            proj_kq2(c)
            proj_v(c)
        pair_part(0, 0, 8)
        proj_k(2)
        proj_q(2)
        proj_kq2(2)
        proj_v(2)
        pair_part(0, 8, 12)
        pair_part(1, 0, 8)
        proj_k(3)
        proj_q(3)
        proj_kq2(3)
        proj_v(3)
        pair_part(0, 12, ST)
        pair_part(1, 8, ST)
        h2_part(0, 0, ST)
        pair_part(2, 0, ST)
        # seq-tiles 0..7 (q columns 0..1023) have all three heads done
        pending = list(range(0, 8))
        for _ in range(4):
            if pending:
                out_proj(pending.pop(0))
        h2_part(1, 0, ST)
        for _ in range(2):
            if pending:
                out_proj(pending.pop(0))
        pair_part(3, 0, ST)
        for st in pending:
            out_proj(st)
        for st in range(8, 16):
            out_proj(st)

        if debug:
            nc.sync.dma_start(out=d_qTA[:, :], in_=qTA)
            nc.sync.dma_start(out=d_qTB[:, :], in_=qTB[64:128, :])
            nc.sync.dma_start(out=d_kTA[:, :], in_=kTA)
            nc.sync.dma_start(out=d_kTB[:, :], in_=kTB[64:128, :])
            nc.sync.dma_start(
                out=d_v[:, :], in_=v_sb.rearrange("p a b c -> p (a b c)")
            )
            nc.sync.dma_start(out=d_ctxA[:, :], in_=ctxA)
            nc.sync.dma_start(out=d_ctxB[:, :], in_=ctxB)

    nc.compile()
    return nc


def _bias_block(bq, bk, col):
    # [128, 4]: col0 = bk heads01, col1 = bk head2 (rows 0:64),
    # col2 = bq heads01 (pre-scaled), col3 = bq head2 at rows 64:128
    blk = np.zeros((128, 4), np.float32)
    blk[:, 0] = bk[col : col + 128]
    blk[0:64, 1] = bk[col + 128 : col + 192]
    blk[:, 2] = bq[col : col + 128] * np.float32(0.125)
    blk[64:128, 3] = bq[col + 128 : col + 192] * np.float32(0.125)
    return blk


def _prep_in_maps(inputs):
    x = np.asarray(inputs["x"], dtype=np.float32)
    Wq = np.asarray(inputs["Wq"], dtype=np.float32)
    Wk = np.asarray(inputs["Wk"], dtype=np.float32)
    Wv = np.asarray(inputs["Wv"], dtype=np.float32)
    Wo = np.asarray(inputs["Wo"], dtype=np.float32)
    bq = np.asarray(inputs["bq"], dtype=np.float32)
    bk = np.asarray(inputs["bk"], dtype=np.float32)

    in_maps = []
    for c in range(NCORES):
        b = c // 4
        col = (c % 4) * DH3
        sl = slice(col, col + DH3)
        in_maps.append(
            {
                "xT": np.ascontiguousarray(x[b].T),
                "wq": np.ascontiguousarray(Wq[:, col : col + 128])
                * np.float32(0.125),
                "wk": np.ascontiguousarray(Wk[:, col : col + 128]),
                "wkq2": np.concatenate(
                    [
                        Wk[:, col + 128 : col + 192],
                        Wq[:, col + 128 : col + 192] * np.float32(0.125),
                    ],
                    axis=1,
                ),
                "wv": np.concatenate(
                    [Wv[:, sl], np.zeros((D, 64), np.float32)], axis=1
                ),
                "wo": np.ascontiguousarray(Wo[sl, :]),
                "bias": _bias_block(bq, bk, col),
            }
        )
    return in_maps


def _combine(results, inputs):
    Wo = np.asarray(inputs["Wo"], dtype=np.float32)
    bv = np.asarray(inputs["bv"], dtype=np.float32)
    bo = np.asarray(inputs["bo"], dtype=np.float32)
    base = bv @ Wo + bo  # [D]
    out = np.empty((B, S, D), dtype=np.float32)
    for b in range(B):
        acc = results[4 * b]["out"].astype(np.float32)
        for c in range(4 * b + 1, 4 * b + 4):
            acc = acc + results[c]["out"]
        out[b] = acc + base
    return out


def run(inputs, trace: bool = False):
    """Run the 8-core kernel; returns (output, BassKernelResults)."""
    global _CACHED_NC
    if _CACHED_NC is None:
        _CACHED_NC = _build_nc()
    in_maps = _prep_in_maps(inputs)
    try:
        res = run_bass_kernel_spmd(
            _CACHED_NC, in_maps, core_ids=list(range(NCORES)), trace=trace
        )
    except ModuleNotFoundError:
        # BASS_TRACE set but the axon NTFF profile hook isn't shipped in
        # this container — retry without tracing.
        import os

        os.environ["BASS_NEVER_TRACE"] = "1"
        res = run_bass_kernel_spmd(
            _CACHED_NC, in_maps, core_ids=list(range(NCORES)), trace=False
        )
    return _combine(res.results, inputs), res


def kernel(**inputs) -> np.ndarray:
    out, _ = run(inputs)
    return out



# revision 2
# speedup vs baseline: 1.0039x; 1.0039x over previous
"""MultiHeadAttention Trainium2 Bass kernel, v2.

Problem: B=2, S=2048, D=768, H=12 heads, head_dim=64.
    q = x@Wq+bq; k = x@Wk+bk; v = x@Wv+bv   (per-head split)
    out = softmax(q k^T / 8) v, heads merged, @ Wo + bo

Sharding (8 cores): core c handles batch b=c//4 and 3 heads (c%4)*3..+3
(Megatron attention: column-split of Wq/Wk/Wv, row-split of Wo). Each core
produces a partial [S, D] output; the host sums the 4 partials per batch and
adds (bv @ Wo + bo) once.

v2 redesign vs v1 (179.3us): the cost model charges a matmul ap_size(out)
cycles regardless of M/K, with NO overlap between matmuls. v1's ctx^T
matmuls ([65, Nq] out) wasted half the PE partition dim. v2 computes ctx in
[q=128, d] orientation (E tile as lhsT, [v|1] as rhs, N=65 per q-subtile):
ctx cost halves (98304 -> 49920 cycles) and the softmax denominator becomes
per-PARTITION (per query), so normalization is a cheap DVE tensor op instead
of the gpsimd partition-broadcast chain. ctx is then PE-transposed (4096
cycles) for the out-projection. The exp stream on ACT (96 x [128,1024] =
~100us) is the hard floor; PE (~97us) hides under it.

dtypes: moving-operand dtype sets matmul speed (f32r needs N>=256 for
1cyc/row; bf16 is 1cyc/row at any N). qT/kT stay f32r (full fp32 data);
x/v/ctx/Wv/Wo/identity are bf16 (small-N matmuls).

kernel(**inputs) takes FULL unsharded inputs and returns the FULL output.
"""

import numpy as np

import concourse.bass as bass
import concourse.mybir as mybir
import concourse.tile as tile
from concourse import bacc
from concourse.bass_utils import run_bass_kernel_spmd

F32 = mybir.dt.float32
F32R = mybir.dt.float32r
BF16 = mybir.dt.bfloat16

B, S, D = 2, 2048, 768
H, DH = 12, 64
NCORES = 8
HPC = 3                # heads per core
DH3 = HPC * DH         # 192 (per-core slice of the model dim)
KT = D // 128          # 6 contraction tiles for D
ST = S // 128          # 16 sequence tiles
GW = 1024              # attention q-chunk (g) width
NG = S // GW           # 2

_CACHED_NC = None


def _build_nc(debug: bool = False) -> bass.Bass:
    nc = bacc.Bacc()

    xT = nc.dram_tensor("xT", [D, S], BF16, kind="ExternalInput")
    wq = nc.dram_tensor("wq", [128, KT * 128], BF16, kind="ExternalInput")
    wk = nc.dram_tensor("wk", [128, KT * 128], BF16, kind="ExternalInput")
    wkq2 = nc.dram_tensor("wkq2", [128, KT * 128], BF16, kind="ExternalInput")
    wv = nc.dram_tensor("wv", [128, KT * DH3], BF16, kind="ExternalInput")
    wo = nc.dram_tensor("wo", [DH3, D], BF16, kind="ExternalInput")
    bias = nc.dram_tensor("bias", [128, 4], F32, kind="ExternalInput")
    ident = nc.dram_tensor("ident", [128, 128], BF16, kind="ExternalInput")
    out = nc.dram_tensor("out", [S, D], BF16, kind="ExternalOutput")
    if debug:
        d_qTA = nc.dram_tensor("d_qTA", [128, S], F32, kind="ExternalOutput")
        d_kTA = nc.dram_tensor("d_kTA", [128, S], F32, kind="ExternalOutput")
        d_qTB = nc.dram_tensor("d_qTB", [128, S], F32, kind="ExternalOutput")
        d_kTB = nc.dram_tensor("d_kTB", [128, S], F32, kind="ExternalOutput")
        d_v = nc.dram_tensor("d_v", [128, ST * HPC * (DH + 1)], F32,
                             kind="ExternalOutput")
        d_ctx = nc.dram_tensor("d_ctx", [128, ST * DH3], F32,
                               kind="ExternalOutput")
        d_ctxTA = nc.dram_tensor("d_ctxTA", [128, S], F32, kind="ExternalOutput")
        d_ctxTB = nc.dram_tensor("d_ctxTB", [64, S], F32, kind="ExternalOutput")

    with (
        tile.TileContext(nc) as tc,
        tc.tile_pool(name="big", bufs=1) as big,
        tc.tile_pool(name="work", bufs=2) as work,
        tc.tile_pool(name="expp", bufs=3) as expp,
        tc.tile_pool(name="outp", bufs=6) as outp,
        tc.tile_pool(name="psS", bufs=2, space="PSUM") as psS,
        tc.tile_pool(name="psB", bufs=2, space="PSUM") as psB,
        tc.tile_pool(name="psF", bufs=2, space="PSUM") as psF,
    ):
        # ---- persistent SBUF tensors ----
        x_sb = big.tile([128, KT, S], BF16)          # xT: [p, ktile, s]
        wq_sb = big.tile([128, KT, 128], BF16)
        wk_sb = big.tile([128, KT, 128], BF16)
        wkq2_sb = big.tile([128, KT, 128], BF16)     # [k_h2 | q_h2]
        wv_sb = big.tile([128, KT, DH3], BF16)
        woA_sb = big.tile([128, D], BF16)            # Wo rows 0..127
        woB_sb = big.tile([64, D], BF16)             # Wo rows 128..191
        bias_sb = big.tile([128, 4], F32)
        ident_sb = big.tile([128, 128], BF16)
        qTA = big.tile([128, S], F32R)               # q^T heads 0,1
        kTA = big.tile([128, S], F32R)
        qTB = big.tile([128, S], F32R)               # head 2 in rows 64:128
        kTB = big.tile([128, S], F32R)
        v_sb = big.tile([128, ST, HPC, DH + 1], BF16)  # v rows + ones col
        ctx_sb = big.tile([128, ST, DH3], BF16)      # [q-part, qt, h*64+d]
        ctxTA = big.tile([128, S], BF16)             # ctx^T heads 0,1
        ctxTB = big.tile([64, S], BF16)              # ctx^T head 2

        # ---- DMA loads ----
        # The DMA bus is effectively serial (~360GB/s) and round-robins
        # across the SP/Pool/ACT queues, so the queue assignment below sets
        # the bus order: wq, wk, bias | x0, x1, wv | x2, x3, wkq2 | wo, ident.
        # single sync queue so the serial DMA bus moves bytes in exactly
        # the order the pipeline consumes them; bias rides the scalar queue
        # (tiny, interleaves once per round-robin round).
        nc.sync.dma_start(out=wq_sb, in_=wq.rearrange("p (kt m) -> p kt m", kt=KT))
        nc.scalar.dma_start(out=bias_sb, in_=bias[:, :])
        for c in range(4):
            cs = slice(c * 256, (c + 1) * 256)
            nc.sync.dma_start(
                out=x_sb[:, :, cs],
                in_=xT[:, cs].rearrange("(kt p) q -> p kt q", p=128),
            )
        nc.sync.dma_start(out=wk_sb, in_=wk.rearrange("p (kt m) -> p kt m", kt=KT))
        nc.sync.dma_start(out=wv_sb, in_=wv.rearrange("p (kt m) -> p kt m", kt=KT))
        nc.sync.dma_start(out=wkq2_sb, in_=wkq2.rearrange("p (kt m) -> p kt m", kt=KT))
        for c in range(2, 4):
            cs = slice(c * 512, (c + 1) * 512)
            nc.sync.dma_start(
                out=x_sb[:, :, cs],
                in_=xT[:, cs].rearrange("(kt p) q -> p kt q", p=128),
            )
        nc.sync.dma_start(out=woA_sb, in_=wo[0:128, :])
        nc.sync.dma_start(out=woB_sb, in_=wo[128:DH3, :])
        nc.sync.dma_start(out=ident_sb, in_=ident[:, :])
        nc.vector.memset(v_sb[:, :, :, DH : DH + 1], 1.0)

        # preload the Exp activation table during the DMA wait (the implicit
        # LoadActFuncSet lands before this dummy, off the critical path)
        dum_in = big.tile([1, 1], F32)
        dum_out = big.tile([1, 1], F32)
        nc.vector.memset(dum_in, 0.0)
        nc.scalar.activation(dum_out, dum_in, mybir.ActivationFunctionType.Exp)

        # warm the PE p-state during the x DMA wait: the Tensor engine ramps
        # 0.65 -> 1.2 -> 2.4 GHz only after ~3us of continuous execution and
        # the ramp resets when PE idles, so burn the DMA wait on throwaway
        # matmuls sized to end right as the first x chunk lands (~6us).
        warm = big.tile([128, 512], BF16)
        nc.vector.memset(warm, 0.0)
        for _ in range(8):
            pw = psS.tile([128, 512], F32, tag="s", name="ps_warm")
            nc.tensor.matmul(pw, lhsT=warm[:, 0:128], rhs=warm, start=True,
                             stop=True)

        # ---- projection helpers ----
        def proj_qk(pool, w_sb, cs, evict):
            n = cs.stop - cs.start
            ps = pool.tile([128, n], F32, tag=("s" if pool is psS else "f"),
                           name="ps_qk", padded_shape=None)
            for kt in range(KT):
                nc.tensor.matmul(
                    ps,
                    lhsT=w_sb[:, kt, :],
                    rhs=x_sb[:, kt, cs],
                    start=(kt == 0),
                    stop=(kt == KT - 1),
                )
            evict(ps, cs)

        def ev_k(ps, cs):
            nc.vector.tensor_scalar_add(kTA[:, cs], ps, bias_sb[:, 0:1])

        def ev_q(ps, cs):
            nc.vector.tensor_scalar_add(qTA[:, cs], ps, bias_sb[:, 2:3])

        def ev_kq2(ps, cs):
            # psum rows 0:64 = k_h2 (up-shift to 64:128), rows 64:128 = q_h2
            nc.vector.tensor_scalar_add(kTB[64:128, cs], ps[0:64, :], bias_sb[0:64, 1:2])
            nc.vector.tensor_scalar_add(qTB[64:128, cs], ps[64:128, :], bias_sb[64:128, 3:4])

        def proj_v(st):
            ss = slice(st * 128, (st + 1) * 128)
            ps_v = psF.tile([128, DH3], F32, tag="f", name="ps_v")
            for kt in range(KT):
                nc.tensor.matmul(
                    ps_v,
                    lhsT=x_sb[:, kt, ss],
                    rhs=wv_sb[:, kt, :],
                    start=(kt == 0),
                    stop=(kt == KT - 1),
                )
            nc.vector.tensor_copy(
                v_sb[:, st, :, 0:DH],
                ps_v.rearrange("p (h d) -> p h d", h=HPC),
            )

        # head h (q/k)^T slices: heads 0,1 in kTA/qTA rows 0:64 / 64:128,
        # head 2 in kTB/qTB rows 64:128.
        def kq_rows(h):
            if h == 0:
                return kTA, qTA, slice(0, 64)
            if h == 1:
                return kTA, qTA, slice(64, 128)
            return kTB, qTB, slice(64, 128)

        # ---- attention pipeline pieces ----
        # stream of (h, q0, qw, j): per unit, j walks 16 key tiles over the
        # q-window [q0, q0+qw). g1 runs h2 first so the head-2 (ctxTB)
        # transpose block is ready early; the LAST unit is split into two
        # 512-wide subunits so the first half of its normalize/transpose/
        # out-projection overlaps the second half's exp stream (shorter
        # serial tail, at the cost of 16 narrower exps).
        units = [(0, 0, GW), (1, 0, GW), (2, 0, GW), (2, GW, GW),
                 (0, GW, GW), (1, GW, 512), (1, GW + 512, 512)]
        seq = [(h, q0, qw, j) for (h, q0, qw) in units for j in range(ST)]

        sc_tiles = {}   # (h, g, j) -> scores psum tile
        ctx_ps = {}     # (h, g, half) -> ctx psum tile

        def sc_step(h, q0, qw, j):
            # 512-wide matmuls: a matmul output must stay within one 2KB
            # PSUM bank (N <= 512 fp32)
            kk, qq, rows = kq_rows(h)
            ps = psS.tile([128, qw], F32, tag="s", name="ps_sc")
            for hs in range(qw // 512):
                nc.tensor.matmul(
                    ps[:, hs * 512 : (hs + 1) * 512],
                    lhsT=kk[rows, j * 128 : (j + 1) * 128],
                    rhs=qq[rows, q0 + hs * 512 : q0 + (hs + 1) * 512],
                    start=True,
                    stop=True,
                )
            sc_tiles[(h, q0, j)] = ps

        def exp_step(h, q0, qw, j):
            ps = sc_tiles.pop((h, q0, j))
            et = expp.tile([128, qw], BF16, tag="e", name="expT")
            nc.scalar.activation(et, ps, mybir.ActivationFunctionType.Exp)
            return et

        def ctx_step(h, q0, qw, j, et):
            for half in range(qw // 512):
                key = (h, q0, half)
                if key not in ctx_ps:
                    ctx_ps[key] = psB.tile([128, 4 * (DH + 1)], F32, tag="b",
                                           name="ps_ctx")
                pc = ctx_ps[key]
                for qq in range(4):
                    # start marks the WHOLE 2KB psum bank pending-zero, so
                    # only the first write of the bank's group may set it
                    # (qq>0 first-writes land on pending bytes = overwrite).
                    qloc = half * 4 + qq
                    nc.tensor.matmul(
                        pc[:, qq * (DH + 1) : (qq + 1) * (DH + 1)],
                        lhsT=et[:, qloc * 128 : (qloc + 1) * 128],
                        rhs=v_sb[:, j, h, :],
                        start=(j == 0 and qq == 0),
                        stop=(j == ST - 1 and qq == 3),
                        skip_group_check=True,
                    )

        def norm_evict(h, q0, half, last=False):
            # psum [128, 4*(65)]: per qq, cols 0:64 = ctx, col 64 = denom.
            pc = ctx_ps.pop((h, q0, half))
            v3 = pc.rearrange("p (qq c) -> p qq c", c=DH + 1)
            den = work.tile([128, 4], F32, tag="den", name="den")
            nc.vector.tensor_copy(den, v3[:, :, DH : DH + 1].squeeze(-1))
            rcp = work.tile([128, 4], F32, tag="rcp", name="rcp")
            nc.vector.reciprocal_approx_fast(out=rcp, in_=den)
            qt0 = q0 // 128 + half * 4
            if last:
                # final unit: ACT is idle after the last exp — normalize
                # there (Copy with per-partition scale), one qq per instr,
                # in parallel with DVE doing the other half
                for qq in range(4):
                    nc.scalar.activation(
                        ctx_sb[:, qt0 + qq, h * DH : (h + 1) * DH],
                        v3[:, qq, 0:DH],
                        mybir.ActivationFunctionType.Copy,
                        scale=rcp[:, qq : qq + 1],
                    )
                return
            nc.vector.tensor_mul(
                ctx_sb[:, qt0 : qt0 + 4, h * DH : (h + 1) * DH],
                v3[:, :, 0:DH],
                rcp.unsqueeze(-1).broadcast_to([128, 4, DH]),
            )

        # ---- transpose + out-projection ----
        def trans_block(g, h, half):
            # ctx [q, d] -> ctx^T [d, q] for one head, 4 q-subtiles, via PE
            # transpose. Per-head so each block is ready right after that
            # head's norm_evict: h0 -> ctxTA rows 0:64, h1 -> ctxTA rows
            # 64:128, h2 -> ctxTB rows 0:64.
            rows = slice(64, 128) if h == 1 else slice(0, 64)
            dst = ctxTB if h == 2 else ctxTA
            pt = psF.tile([128, 512], BF16, tag="f", name="ps_t")
            for qq in range(4):
                qt = g * 8 + half * 4 + qq
                nc.tensor.transpose(
                    pt[rows, qq * 128 : (qq + 1) * 128],
                    ctx_sb[:, qt, h * DH : (h + 1) * DH], ident_sb,
                )
            cs = slice(g * GW + half * 512, g * GW + (half + 1) * 512)
            nc.vector.tensor_copy(dst[rows if h == 1 else slice(0, 64), cs],
                                  pt[rows, :])

        o_tiles = {}

        def outproj_chunk(qt, c, pool, tag):
            # c=0: cols 0:512, c=1: cols 512:768 (via psF filler pool);
            # pool=psS at the tail does the full row in one [128, 768] tile.
            if pool is psS:
                osl = slice(0, D)
            else:
                osl = slice(c * 512, 512 if c == 0 else D)
            n = osl.stop - osl.start
            po = pool.tile([128, n], F32, tag=tag, name="ps_o")
            nc.tensor.matmul(
                po, lhsT=ctxTA[:, qt * 128 : (qt + 1) * 128],
                rhs=woA_sb[:, osl], start=True, stop=False,
            )
            nc.tensor.matmul(
                po, lhsT=ctxTB[:, qt * 128 : (qt + 1) * 128],
                rhs=woB_sb[:, osl], start=False, stop=True,
            )
            ss = slice(qt * 128, (qt + 1) * 128)
            if qt not in o_tiles:
                o_tiles[qt] = outp.tile([128, D], BF16, tag="o", name="o_sb")
            o_sb = o_tiles[qt]
            if pool is psS and tag == "act":
                # tail odd tiles: evict on the (post-exp idle) ACT engine
                nc.scalar.activation(o_sb[:, osl], po,
                                     mybir.ActivationFunctionType.Copy)
            else:
                nc.vector.tensor_copy(o_sb[:, osl], po)
            if osl.stop == D:
                nc.sync.dma_start(out=out[ss, :], in_=o_sb)

        def outproj_tail(qt, kind):
            # kind 0: one [128, 768] psS tile, DVE evict, sync DMA.
            # kind 1: two psF chunks, ACT Copy evicts, Pool-queue DMA.
            # Alternating kinds gives 4 psum tiles and 2 evict engines in
            # flight, so the tail streams at matmul rate.
            ss = slice(qt * 128, (qt + 1) * 128)
            o_sb = outp.tile([128, D], BF16, tag="o", name="o_sb")
            if kind == 0:
                po = psS.tile([128, D], F32, tag="s", name="ps_o")
            for osl in (slice(0, 512), slice(512, D)):
                if kind == 0:
                    pr = po[:, osl]
                else:
                    pr = psF.tile([128, osl.stop - osl.start], F32, tag="f",
                                  name="ps_o")
                nc.tensor.matmul(
                    pr, lhsT=ctxTA[:, qt * 128 : (qt + 1) * 128],
                    rhs=woA_sb[:, osl], start=True, stop=False,
                )
                nc.tensor.matmul(
                    pr, lhsT=ctxTB[:, qt * 128 : (qt + 1) * 128],
                    rhs=woB_sb[:, osl], start=False, stop=True,
                )
                if kind == 1:
                    nc.scalar.activation(o_sb[:, osl], pr,
                                         mybir.ActivationFunctionType.Copy)
            if kind == 0:
                nc.vector.tensor_copy(o_sb, po)
            nc.sync.dma_start(out=out[ss, :], in_=o_sb)

        def outproj_last(qt):
            # final tile: halves evicted concurrently on DVE and ACT into
            # separate tiles, each DMA'd immediately — shortens the final
            # evict->DMA->sem chain that nothing can overlap.
            ss = slice(qt * 128, (qt + 1) * 128)
            po = psS.tile([128, D], F32, tag="s", name="ps_o")
            for osl in (slice(0, 512), slice(512, D)):
                nc.tensor.matmul(
                    po[:, osl], lhsT=ctxTA[:, qt * 128 : (qt + 1) * 128],
                    rhs=woA_sb[:, osl], start=True, stop=False,
                )
                nc.tensor.matmul(
                    po[:, osl], lhsT=ctxTB[:, qt * 128 : (qt + 1) * 128],
                    rhs=woB_sb[:, osl], start=False, stop=True,
                )
            oa = outp.tile([128, 384], BF16, tag="oa", name="oa_sb")
            ob = outp.tile([128, 384], BF16, tag="ob", name="ob_sb")
            nc.vector.tensor_copy(oa, po[:, 0:384])
            nc.scalar.activation(ob, po[:, 384:D],
                                 mybir.ActivationFunctionType.Copy)
            nc.sync.dma_start(out=out[ss, 0:384], in_=oa)
            nc.sync.dma_start(out=out[ss, 384:D], in_=ob)

        # ---- startup: projections needed before the exp stream starts ----
        # q g0 in 256-col pieces pipelined behind the x DMA pieces; k cols
        # 0:384 cover sc j=0..2 (the rest stream in as fillers). sc(0)/sc(1)
        # emitted as early as their operands allow — the v tiles (only
        # needed by ctx) come after.
        for c in range(4):
            proj_qk(psS, wq_sb, slice(c * 256, (c + 1) * 256), ev_q)
        proj_qk(psS, wk_sb, slice(0, 128), ev_k)

        # ---- filler queues (each item <= ~480ns of PE, emitted one per
        # pipeline step so sc/ctx never starve the ACT exp stream) ----
        def mk_qk(w_sb, cs, evict):
            return lambda: proj_qk(psF, w_sb, cs, evict)

        # g0 phase: remaining k (128-col chunks, needed at sc(*,j) col j*128),
        # v tiles (needed at ctx(h0, j=st)), kq2 (needed by unit (h2, g0)).
        kcs = [slice(384 + 128 * i, 384 + 128 * (i + 1)) for i in range(13)]
        k2cs = [slice(128 * i, 128 * (i + 1)) for i in range(16)]
        qcs = [slice(1024 + 128 * i, 1024 + 128 * (i + 1)) for i in range(8)]
        fill_g0 = [lambda: proj_v(2), lambda: proj_v(3),
                   mk_qk(wk_sb, kcs[0], ev_k)]
        for i in range(12):
            fill_g0.append(mk_qk(wk_sb, kcs[i + 1], ev_k))
            fill_g0.append(lambda st=i + 4: proj_v(st))
        fill_g0 += [mk_qk(wkq2_sb, cs, ev_kq2) for cs in k2cs]

        # g1 phase (units h2, h0, h1): q g1 chunks first (needed by unit
        # (h0, g1)), then g0 transposes + out-projections, then the g1
        # transpose blocks for h2 (ready after unit (2,1)) and h0 (ready
        # after (0,1)) — only h1's block is left for the tail.
        fill_g1 = [mk_qk(wq_sb, cs, ev_q) for cs in qcs]
        for h in range(HPC):
            for half in range(2):
                fill_g1.append(lambda h=h, half=half: trans_block(0, h, half))
        for qt in range(8):
            fill_g1.append(lambda qt=qt: outproj_chunk(qt, 0, psF, "f"))
            fill_g1.append(lambda qt=qt: outproj_chunk(qt, 1, psF, "f"))
        for half in range(2):
            fill_g1.append(lambda half=half: trans_block(1, 2, half))
        for half in range(2):
            fill_g1.append(lambda half=half: trans_block(1, 0, half))

        fillers = {0: fill_g0, 1: fill_g1}

        # ---- main pipelined emission ----
        sc_step(*seq[0])
        proj_qk(psF, wk_sb, slice(128, 256), ev_k)
        sc_step(*seq[1])
        proj_qk(psF, wk_sb, slice(256, 384), ev_k)
        proj_v(0)
        proj_v(1)
        for i, (h, g, j) in enumerate(seq):
            et = exp_step(h, g, j)
            ctx_step(h, g, j, et)
            if i + 2 < len(seq):
                sc_step(*seq[i + 2])
            if j == ST - 1:
                norm_evict(h, g, 0)
                norm_evict(h, g, 1)
            elif j < ST - 2:
                # no filler in the last two steps of a unit: keeps the DVE
                # queue clear so norm_evict frees the psB slots in time for
                # the next unit's first ctx matmuls.
                fq = fillers[g]
                if fq:
                    fq.pop(0)()

        # drain any unemitted fillers
        for fq in (fill_g0, fill_g1):
            while fq:
                fq.pop(0)()

        # ---- tail: transpose the h1 block of g1 + out-proj qt 8..15 ----
        trans_block(1, 1, 0)
        trans_block(1, 1, 1)
        for qt in range(8, 16):
            outproj_tail(qt, qt % 2)

        if debug:
            nc.sync.dma_start(out=d_qTA[:, :], in_=qTA.bitcast(F32))
            nc.sync.dma_start(out=d_kTA[:, :], in_=kTA.bitcast(F32))
            nc.sync.dma_start(out=d_qTB[:, :], in_=qTB.bitcast(F32))
            nc.sync.dma_start(out=d_kTB[:, :], in_=kTB.bitcast(F32))
            vf = work.tile([128, ST * HPC * (DH + 1)], F32, tag="dv", name="vf")
            nc.vector.tensor_copy(vf, v_sb.rearrange("p a b c -> p (a b c)"))
            nc.sync.dma_start(out=d_v[:, :], in_=vf)
            cf = work.tile([128, ST * DH3], F32, tag="dc", name="cf")
            nc.vector.tensor_copy(cf, ctx_sb.rearrange("p a b -> p (a b)"))
            nc.sync.dma_start(out=d_ctx[:, :], in_=cf)
            caf = work.tile([128, S], F32, tag="dca", name="caf")
            nc.vector.tensor_copy(caf, ctxTA)
            nc.sync.dma_start(out=d_ctxTA[:, :], in_=caf)
            cbf = work.tile([64, S], F32, tag="dcb", name="cbf")
            nc.vector.tensor_copy(cbf, ctxTB)
            nc.sync.dma_start(out=d_ctxTB[:, :], in_=cbf)

    nc.compile()
    return nc


def _w_rearrange(w):
    """[768, M] -> [128, 6*M] bf16: row p holds w[kt*128+p, :] for kt=0..5,
    so the device DMA is one contiguous segment per partition."""
    import ml_dtypes

    d, m = w.shape
    kt = d // 128
    return np.ascontiguousarray(
        w.reshape(kt, 128, m).transpose(1, 0, 2).reshape(128, kt * m)
    ).astype(ml_dtypes.bfloat16)


def _bias_block(bq, bk, col):
    # [128, 4]: col0 = bk heads01, col1 = bk head2 (rows 0:64),
    # col2 = bq heads01 (pre-scaled), col3 = bq head2 at rows 64:128
    blk = np.zeros((128, 4), np.float32)
    blk[:, 0] = bk[col : col + 128]
    blk[0:64, 1] = bk[col + 128 : col + 192]
    blk[:, 2] = bq[col : col + 128] * np.float32(0.125)
    blk[64:128, 3] = bq[col + 128 : col + 192] * np.float32(0.125)
    return blk


def _prep_in_maps(inputs):
    import ml_dtypes

    bf16 = ml_dtypes.bfloat16
    x = np.asarray(inputs["x"], dtype=np.float32)
    Wq = np.asarray(inputs["Wq"], dtype=np.float32)
    Wk = np.asarray(inputs["Wk"], dtype=np.float32)
    Wv = np.asarray(inputs["Wv"], dtype=np.float32)
    Wo = np.asarray(inputs["Wo"], dtype=np.float32)
    bq = np.asarray(inputs["bq"], dtype=np.float32)
    bk = np.asarray(inputs["bk"], dtype=np.float32)
    ident = np.eye(128, dtype=np.float32).astype(bf16)

    in_maps = []
    for c in range(NCORES):
        b = c // 4
        col = (c % 4) * DH3
        sl = slice(col, col + DH3)
        in_maps.append(
            {
                "xT": np.ascontiguousarray(x[b].T).astype(bf16),
                "wq": _w_rearrange(Wq[:, col : col + 128] * np.float32(0.125)),
                "wk": _w_rearrange(Wk[:, col : col + 128]),
                "wkq2": _w_rearrange(np.concatenate(
                    [
                        Wk[:, col + 128 : col + 192],
                        Wq[:, col + 128 : col + 192] * np.float32(0.125),
                    ],
                    axis=1,
                )),
                "wv": _w_rearrange(Wv[:, sl]),
                "wo": np.ascontiguousarray(Wo[sl, :]).astype(bf16),
                "bias": _bias_block(bq, bk, col),
                "ident": ident,
            }
        )
    return in_maps


def _combine(results, inputs):
    Wo = np.asarray(inputs["Wo"], dtype=np.float32)
    bv = np.asarray(inputs["bv"], dtype=np.float32)
    bo = np.asarray(inputs["bo"], dtype=np.float32)
    base = bv @ Wo + bo  # [D]
    out = np.empty((B, S, D), dtype=np.float32)
    for b in range(B):
        acc = results[4 * b]["out"].astype(np.float32)
        for c in range(4 * b + 1, 4 * b + 4):
            acc = acc + results[c]["out"].astype(np.float32)
        out[b] = acc + base
    return out


def run(inputs, trace: bool = False):
    """Run the 8-core kernel; returns (output, BassKernelResults)."""
    global _CACHED_NC
    if _CACHED_NC is None:
        _CACHED_NC = _build_nc()
    in_maps = _prep_in_maps(inputs)
    try:
        res = run_bass_kernel_spmd(
            _CACHED_NC, in_maps, core_ids=list(range(NCORES)), trace=trace
        )
    except ModuleNotFoundError:
        import os

        os.environ["BASS_NEVER_TRACE"] = "1"
        res = run_bass_kernel_spmd(
            _CACHED_NC, in_maps, core_ids=list(range(NCORES)), trace=False
        )
    return _combine(res.results, inputs), res


def kernel(**inputs) -> np.ndarray:
    out, _ = run(inputs)
    return out


# revision 3
# speedup vs baseline: 1.0056x; 1.0017x over previous
"""MultiHeadAttention Trainium2 Bass kernel, v2.

Problem: B=2, S=2048, D=768, H=12 heads, head_dim=64.
    q = x@Wq+bq; k = x@Wk+bk; v = x@Wv+bv   (per-head split)
    out = softmax(q k^T / 8) v, heads merged, @ Wo + bo

Sharding (8 cores): core c handles batch b=c//4 and 3 heads (c%4)*3..+3
(Megatron attention: column-split of Wq/Wk/Wv, row-split of Wo). Each core
produces a partial [S, D] output; the host sums the 4 partials per batch and
adds (bv @ Wo + bo) once.

Timeline: 126.8us vs v1's 179.3us (1.41x). ~10.5us DMA-bound startup,
~100us ACT exp stream (the floor: 96 x [128,1024] exps at 1 col/cycle,
1.2GHz), ~12.5us tail (last unit's norm -> transpose -> out-proj -> DMA).

v2 redesign vs v1 (179.3us): the cost model charges a matmul ap_size(out)
cycles regardless of M/K, with NO overlap between matmuls. v1's ctx^T
matmuls ([65, Nq] out) wasted half the PE partition dim. v2 computes ctx in
[q=128, d] orientation (E tile as lhsT, [v|1] as rhs, N=65 per q-subtile):
ctx cost halves (98304 -> 49920 cycles) and the softmax denominator becomes
per-PARTITION (per query), so normalization is a cheap DVE tensor op instead
of the gpsimd partition-broadcast chain. ctx is then PE-transposed (4096
cycles) for the out-projection. The exp stream on ACT (96 x [128,1024] =
~100us) is the hard floor; PE (~97us) hides under it.

dtypes: moving-operand dtype sets matmul speed (f32r needs N>=256 for
1cyc/row; bf16 is 1cyc/row at any N). qT/kT stay f32r (full fp32 data);
x/v/ctx/Wv/Wo/identity are bf16 (small-N matmuls).

kernel(**inputs) takes FULL unsharded inputs and returns the FULL output.
"""

import numpy as np

import concourse.bass as bass
import concourse.mybir as mybir
import concourse.tile as tile
from concourse import bacc
from concourse.bass_utils import run_bass_kernel_spmd

F32 = mybir.dt.float32
F32R = mybir.dt.float32r
BF16 = mybir.dt.bfloat16

B, S, D = 2, 2048, 768
H, DH = 12, 64
NCORES = 8
HPC = 3                # heads per core
DH3 = HPC * DH         # 192 (per-core slice of the model dim)
KT = D // 128          # 6 contraction tiles for D
ST = S // 128          # 16 sequence tiles
GW = 1024              # attention q-chunk (g) width
NG = S // GW           # 2

_CACHED_NC = None


def _build_nc(debug: bool = False) -> bass.Bass:
    nc = bacc.Bacc()

    xT = nc.dram_tensor("xT", [D, S], BF16, kind="ExternalInput")
    wq = nc.dram_tensor("wq", [128, KT * 128], BF16, kind="ExternalInput")
    wk = nc.dram_tensor("wk", [128, KT * 128], BF16, kind="ExternalInput")
    wkq2 = nc.dram_tensor("wkq2", [128, KT * 128], BF16, kind="ExternalInput")
    wv = nc.dram_tensor("wv", [128, KT * DH3], BF16, kind="ExternalInput")
    wo = nc.dram_tensor("wo", [DH3, D], BF16, kind="ExternalInput")
    bias = nc.dram_tensor("bias", [128, 4], F32, kind="ExternalInput")
    ident = nc.dram_tensor("ident", [128, 128], BF16, kind="ExternalInput")
    out = nc.dram_tensor("out", [S, D], BF16, kind="ExternalOutput")
    if debug:
        d_qTA = nc.dram_tensor("d_qTA", [128, S], F32, kind="ExternalOutput")
        d_kTA = nc.dram_tensor("d_kTA", [128, S], F32, kind="ExternalOutput")
        d_qTB = nc.dram_tensor("d_qTB", [128, S], F32, kind="ExternalOutput")
        d_kTB = nc.dram_tensor("d_kTB", [128, S], F32, kind="ExternalOutput")
        d_v = nc.dram_tensor("d_v", [128, ST * HPC * (DH + 1)], F32,
                             kind="ExternalOutput")
        d_ctx = nc.dram_tensor("d_ctx", [128, ST * DH3], F32,
                               kind="ExternalOutput")
        d_ctxTA = nc.dram_tensor("d_ctxTA", [128, S], F32, kind="ExternalOutput")
        d_ctxTB = nc.dram_tensor("d_ctxTB", [64, S], F32, kind="ExternalOutput")

    with (
        tile.TileContext(nc) as tc,
        tc.tile_pool(name="big", bufs=1) as big,
        tc.tile_pool(name="work", bufs=2) as work,
        tc.tile_pool(name="expp", bufs=3) as expp,
        tc.tile_pool(name="outp", bufs=6) as outp,
        tc.tile_pool(name="psS", bufs=2, space="PSUM") as psS,
        tc.tile_pool(name="psB", bufs=2, space="PSUM") as psB,
        tc.tile_pool(name="psF", bufs=2, space="PSUM") as psF,
    ):
        # ---- persistent SBUF tensors ----
        x_sb = big.tile([128, KT, S], BF16)          # xT: [p, ktile, s]
        wq_sb = big.tile([128, KT, 128], BF16)
        wk_sb = big.tile([128, KT, 128], BF16)
        wkq2_sb = big.tile([128, KT, 128], BF16)     # [k_h2 | q_h2]
        wv_sb = big.tile([128, KT, DH3], BF16)
        woA_sb = big.tile([128, D], BF16)            # Wo rows 0..127
        woB_sb = big.tile([64, D], BF16)             # Wo rows 128..191
        bias_sb = big.tile([128, 4], F32)
        ident_sb = big.tile([128, 128], BF16)
        qTA = big.tile([128, S], F32R)               # q^T heads 0,1
        kTA = big.tile([128, S], F32R)
        qTB = big.tile([128, S], F32R)               # head 2 in rows 64:128
        kTB = big.tile([128, S], F32R)
        v_sb = big.tile([128, ST, HPC, DH + 1], BF16)  # v rows + ones col
        ctx_sb = big.tile([128, ST, DH3], BF16)      # [q-part, qt, h*64+d]
        ctxTA = big.tile([128, S], BF16)             # ctx^T heads 0,1
        ctxTB = big.tile([64, S], BF16)              # ctx^T head 2

        # ---- DMA loads ----
        # The DMA bus is effectively serial (~360GB/s) and round-robins
        # across the SP/Pool/ACT queues, so the queue assignment below sets
        # the bus order: wq, wk, bias | x0, x1, wv | x2, x3, wkq2 | wo, ident.
        # single sync queue so the serial DMA bus moves bytes in exactly
        # the order the pipeline consumes them; bias rides the scalar queue
        # (tiny, interleaves once per round-robin round).
        nc.sync.dma_start(out=wq_sb, in_=wq.rearrange("p (kt m) -> p kt m", kt=KT))
        nc.scalar.dma_start(out=bias_sb, in_=bias[:, :])
        for c in range(4):
            cs = slice(c * 256, (c + 1) * 256)
            nc.sync.dma_start(
                out=x_sb[:, :, cs],
                in_=xT[:, cs].rearrange("(kt p) q -> p kt q", p=128),
            )
        nc.sync.dma_start(out=wk_sb, in_=wk.rearrange("p (kt m) -> p kt m", kt=KT))
        nc.sync.dma_start(out=wv_sb, in_=wv.rearrange("p (kt m) -> p kt m", kt=KT))
        nc.sync.dma_start(out=wkq2_sb, in_=wkq2.rearrange("p (kt m) -> p kt m", kt=KT))
        for c in range(2, 4):
            cs = slice(c * 512, (c + 1) * 512)
            nc.sync.dma_start(
                out=x_sb[:, :, cs],
                in_=xT[:, cs].rearrange("(kt p) q -> p kt q", p=128),
            )
        nc.sync.dma_start(out=woA_sb, in_=wo[0:128, :])
        nc.sync.dma_start(out=woB_sb, in_=wo[128:DH3, :])
        nc.sync.dma_start(out=ident_sb, in_=ident[:, :])
        nc.vector.memset(v_sb[:, :, :, DH : DH + 1], 1.0)

        # preload the Exp activation table during the DMA wait (the implicit
        # LoadActFuncSet lands before this dummy, off the critical path)
        dum_in = big.tile([1, 1], F32)
        dum_out = big.tile([1, 1], F32)
        nc.vector.memset(dum_in, 0.0)
        nc.scalar.activation(dum_out, dum_in, mybir.ActivationFunctionType.Exp)

        # warm the PE p-state during the x DMA wait: the Tensor engine ramps
        # 0.65 -> 1.2 -> 2.4 GHz only after ~3us of continuous execution and
        # the ramp resets when PE idles, so burn the DMA wait on throwaway
        # matmuls sized to end right as the first x chunk lands (~6us).
        warm = big.tile([128, 512], BF16)
        nc.vector.memset(warm, 0.0)
        for _ in range(8):
            pw = psS.tile([128, 512], F32, tag="s", name="ps_warm")
            nc.tensor.matmul(pw, lhsT=warm[:, 0:128], rhs=warm, start=True,
                             stop=True)

        # ---- projection helpers ----
        def proj_qk(pool, w_sb, cs, evict):
            n = cs.stop - cs.start
            ps = pool.tile([128, n], F32, tag=("s" if pool is psS else "f"),
                           name="ps_qk", padded_shape=None)
            for kt in range(KT):
                nc.tensor.matmul(
                    ps,
                    lhsT=w_sb[:, kt, :],
                    rhs=x_sb[:, kt, cs],
                    start=(kt == 0),
                    stop=(kt == KT - 1),
                )
            evict(ps, cs)

        def ev_k(ps, cs):
            nc.vector.tensor_scalar_add(kTA[:, cs], ps, bias_sb[:, 0:1])

        def ev_q(ps, cs):
            nc.vector.tensor_scalar_add(qTA[:, cs], ps, bias_sb[:, 2:3])

        def ev_kq2(ps, cs):
            # psum rows 0:64 = k_h2 (up-shift to 64:128), rows 64:128 = q_h2
            nc.vector.tensor_scalar_add(kTB[64:128, cs], ps[0:64, :], bias_sb[0:64, 1:2])
            nc.vector.tensor_scalar_add(qTB[64:128, cs], ps[64:128, :], bias_sb[64:128, 3:4])

        def proj_v(st):
            ss = slice(st * 128, (st + 1) * 128)
            ps_v = psF.tile([128, DH3], F32, tag="f", name="ps_v")
            for kt in range(KT):
                nc.tensor.matmul(
                    ps_v,
                    lhsT=x_sb[:, kt, ss],
                    rhs=wv_sb[:, kt, :],
                    start=(kt == 0),
                    stop=(kt == KT - 1),
                )
            nc.vector.tensor_copy(
                v_sb[:, st, :, 0:DH],
                ps_v.rearrange("p (h d) -> p h d", h=HPC),
            )

        # head h (q/k)^T slices: heads 0,1 in kTA/qTA rows 0:64 / 64:128,
        # head 2 in kTB/qTB rows 64:128.
        def kq_rows(h):
            if h == 0:
                return kTA, qTA, slice(0, 64)
            if h == 1:
                return kTA, qTA, slice(64, 128)
            return kTB, qTB, slice(64, 128)

        # ---- attention pipeline pieces ----
        # stream of (h, q0, qw, j): per unit, j walks 16 key tiles over the
        # q-window [q0, q0+qw). g1 runs h2 first so the head-2 (ctxTB)
        # transpose block is ready early; the LAST unit is split into two
        # 512-wide subunits so the first half of its normalize/transpose/
        # out-projection overlaps the second half's exp stream (shorter
        # serial tail, at the cost of 16 narrower exps).
        units = [(0, 0, GW), (1, 0, GW), (2, 0, GW), (2, GW, GW),
                 (0, GW, GW), (1, GW, 512), (1, GW + 512, 512)]
        seq = [(h, q0, qw, j) for (h, q0, qw) in units for j in range(ST)]

        sc_tiles = {}   # (h, g, j) -> scores psum tile
        ctx_ps = {}     # (h, g, half) -> ctx psum tile

        def sc_step(h, q0, qw, j):
            # 512-wide matmuls: a matmul output must stay within one 2KB
            # PSUM bank (N <= 512 fp32)
            kk, qq, rows = kq_rows(h)
            ps = psS.tile([128, qw], F32, tag="s", name="ps_sc")
            for hs in range(qw // 512):
                nc.tensor.matmul(
                    ps[:, hs * 512 : (hs + 1) * 512],
                    lhsT=kk[rows, j * 128 : (j + 1) * 128],
                    rhs=qq[rows, q0 + hs * 512 : q0 + (hs + 1) * 512],
                    start=True,
                    stop=True,
                )
            sc_tiles[(h, q0, j)] = ps

        def exp_step(h, q0, qw, j):
            ps = sc_tiles.pop((h, q0, j))
            et = expp.tile([128, qw], BF16, tag="e", name="expT")
            nc.scalar.activation(et, ps, mybir.ActivationFunctionType.Exp)
            return et

        def ctx_step(h, q0, qw, j, et):
            for half in range(qw // 512):
                key = (h, q0, half)
                if key not in ctx_ps:
                    ctx_ps[key] = psB.tile([128, 4 * (DH + 1)], F32, tag="b",
                                           name="ps_ctx")
                pc = ctx_ps[key]
                for qq in range(4):
                    # start marks the WHOLE 2KB psum bank pending-zero, so
                    # only the first write of the bank's group may set it
                    # (qq>0 first-writes land on pending bytes = overwrite).
                    qloc = half * 4 + qq
                    nc.tensor.matmul(
                        pc[:, qq * (DH + 1) : (qq + 1) * (DH + 1)],
                        lhsT=et[:, qloc * 128 : (qloc + 1) * 128],
                        rhs=v_sb[:, j, h, :],
                        start=(j == 0 and qq == 0),
                        stop=(j == ST - 1 and qq == 3),
                        skip_group_check=True,
                    )

        def norm_evict(h, q0, half, last=False):
            # psum [128, 4*(65)]: per qq, cols 0:64 = ctx, col 64 = denom.
            pc = ctx_ps.pop((h, q0, half))
            v3 = pc.rearrange("p (qq c) -> p qq c", c=DH + 1)
            den = work.tile([128, 4], F32, tag="den", name="den")
            nc.vector.tensor_copy(den, v3[:, :, DH : DH + 1].squeeze(-1))
            rcp = work.tile([128, 4], F32, tag="rcp", name="rcp")
            nc.vector.reciprocal_approx_fast(out=rcp, in_=den)
            qt0 = q0 // 128 + half * 4
            if last:
                # final unit: ACT is idle after the last exp — normalize
                # there (Copy with per-partition scale), one qq per instr,
                # in parallel with DVE doing the other half
                for qq in range(4):
                    nc.scalar.activation(
                        ctx_sb[:, qt0 + qq, h * DH : (h + 1) * DH],
                        v3[:, qq, 0:DH],
                        mybir.ActivationFunctionType.Copy,
                        scale=rcp[:, qq : qq + 1],
                    )
                return
            nc.vector.tensor_mul(
                ctx_sb[:, qt0 : qt0 + 4, h * DH : (h + 1) * DH],
                v3[:, :, 0:DH],
                rcp.unsqueeze(-1).broadcast_to([128, 4, DH]),
            )

        # ---- transpose + out-projection ----
        def trans_block(g, h, half):
            # ctx [q, d] -> ctx^T [d, q] for one head, 4 q-subtiles, via PE
            # transpose. Per-head so each block is ready right after that
            # head's norm_evict: h0 -> ctxTA rows 0:64, h1 -> ctxTA rows
            # 64:128, h2 -> ctxTB rows 0:64.
            rows = slice(64, 128) if h == 1 else slice(0, 64)
            dst = ctxTB if h == 2 else ctxTA
            pt = psF.tile([128, 512], BF16, tag="f", name="ps_t")
            for qq in range(4):
                qt = g * 8 + half * 4 + qq
                nc.tensor.transpose(
                    pt[rows, qq * 128 : (qq + 1) * 128],
                    ctx_sb[:, qt, h * DH : (h + 1) * DH], ident_sb,
                )
            cs = slice(g * GW + half * 512, g * GW + (half + 1) * 512)
            nc.vector.tensor_copy(dst[rows if h == 1 else slice(0, 64), cs],
                                  pt[rows, :])

        o_tiles = {}

        def outproj_chunk(qt, c, pool, tag):
            # c=0: cols 0:512, c=1: cols 512:768 (via psF filler pool);
            # pool=psS at the tail does the full row in one [128, 768] tile.
            if pool is psS:
                osl = slice(0, D)
            else:
                osl = slice(c * 512, 512 if c == 0 else D)
            n = osl.stop - osl.start
            po = pool.tile([128, n], F32, tag=tag, name="ps_o")
            nc.tensor.matmul(
                po, lhsT=ctxTA[:, qt * 128 : (qt + 1) * 128],
                rhs=woA_sb[:, osl], start=True, stop=False,
            )
            nc.tensor.matmul(
                po, lhsT=ctxTB[:, qt * 128 : (qt + 1) * 128],
                rhs=woB_sb[:, osl], start=False, stop=True,
            )
            ss = slice(qt * 128, (qt + 1) * 128)
            if qt not in o_tiles:
                o_tiles[qt] = outp.tile([128, D], BF16, tag="o", name="o_sb")
            o_sb = o_tiles[qt]
            if pool is psS and tag == "act":
                # tail odd tiles: evict on the (post-exp idle) ACT engine
                nc.scalar.activation(o_sb[:, osl], po,
                                     mybir.ActivationFunctionType.Copy)
            else:
                nc.vector.tensor_copy(o_sb[:, osl], po)
            if osl.stop == D:
                nc.sync.dma_start(out=out[ss, :], in_=o_sb)

        def outproj_tail(qt, kind):
            # kind 0: one [128, 768] psS tile, DVE evict, sync DMA.
            # kind 1: two psF chunks, ACT Copy evicts, Pool-queue DMA.
            # Alternating kinds gives 4 psum tiles and 2 evict engines in
            # flight, so the tail streams at matmul rate.
            ss = slice(qt * 128, (qt + 1) * 128)
            o_sb = outp.tile([128, D], BF16, tag="o", name="o_sb")
            if kind == 0:
                po = psS.tile([128, D], F32, tag="s", name="ps_o")
            for osl in (slice(0, 512), slice(512, D)):
                if kind == 0:
                    pr = po[:, osl]
                else:
                    pr = psF.tile([128, osl.stop - osl.start], F32, tag="f",
                                  name="ps_o")
                nc.tensor.matmul(
                    pr, lhsT=ctxTA[:, qt * 128 : (qt + 1) * 128],
                    rhs=woA_sb[:, osl], start=True, stop=False,
                )
                nc.tensor.matmul(
                    pr, lhsT=ctxTB[:, qt * 128 : (qt + 1) * 128],
                    rhs=woB_sb[:, osl], start=False, stop=True,
                )
                if kind == 1:
                    nc.scalar.activation(o_sb[:, osl], pr,
                                         mybir.ActivationFunctionType.Copy)
            if kind == 0:
                nc.vector.tensor_copy(o_sb, po)
            nc.sync.dma_start(out=out[ss, :], in_=o_sb)

        def outproj_last(qt):
            # final tile: halves evicted concurrently on DVE and ACT into
            # separate tiles, each DMA'd immediately — shortens the final
            # evict->DMA->sem chain that nothing can overlap.
            ss = slice(qt * 128, (qt + 1) * 128)
            po = psS.tile([128, D], F32, tag="s", name="ps_o")
            for osl in (slice(0, 512), slice(512, D)):
                nc.tensor.matmul(
                    po[:, osl], lhsT=ctxTA[:, qt * 128 : (qt + 1) * 128],
                    rhs=woA_sb[:, osl], start=True, stop=False,
                )
                nc.tensor.matmul(
                    po[:, osl], lhsT=ctxTB[:, qt * 128 : (qt + 1) * 128],
                    rhs=woB_sb[:, osl], start=False, stop=True,
                )
            oa = outp.tile([128, 384], BF16, tag="oa", name="oa_sb")
            ob = outp.tile([128, 384], BF16, tag="ob", name="ob_sb")
            nc.vector.tensor_copy(oa, po[:, 0:384])
            nc.scalar.activation(ob, po[:, 384:D],
                                 mybir.ActivationFunctionType.Copy)
            nc.sync.dma_start(out=out[ss, 0:384], in_=oa)
            nc.sync.dma_start(out=out[ss, 384:D], in_=ob)

        # ---- startup: projections needed before the exp stream starts ----
        # q g0 in 256-col pieces pipelined behind the x DMA pieces; k cols
        # 0:384 cover sc j=0..2 (the rest stream in as fillers). sc(0)/sc(1)
        # emitted as early as their operands allow — the v tiles (only
        # needed by ctx) come after.
        for c in range(4):
            proj_qk(psS, wq_sb, slice(c * 256, (c + 1) * 256), ev_q)
        proj_qk(psS, wk_sb, slice(0, 128), ev_k)

        # ---- filler queues (each item <= ~480ns of PE, emitted one per
        # pipeline step so sc/ctx never starve the ACT exp stream) ----
        def mk_qk(w_sb, cs, evict):
            return lambda: proj_qk(psF, w_sb, cs, evict)

        # g0 phase: remaining k (128-col chunks, needed at sc(*,j) col j*128),
        # v tiles (needed at ctx(h0, j=st)), kq2 (needed by unit (h2, g0)).
        kcs = [slice(384 + 128 * i, 384 + 128 * (i + 1)) for i in range(13)]
        k2cs = [slice(128 * i, 128 * (i + 1)) for i in range(16)]
        qcs = [slice(1024 + 128 * i, 1024 + 128 * (i + 1)) for i in range(8)]
        fill_g0 = [lambda: proj_v(2), lambda: proj_v(3),
                   mk_qk(wk_sb, kcs[0], ev_k)]
        for i in range(12):
            fill_g0.append(mk_qk(wk_sb, kcs[i + 1], ev_k))
            fill_g0.append(lambda st=i + 4: proj_v(st))
        fill_g0 += [mk_qk(wkq2_sb, cs, ev_kq2) for cs in k2cs]

        # g1 phase (units h2, h0, h1): q g1 chunks first (needed by unit
        # (h0, g1)), then g0 transposes + out-projections, then the g1
        # transpose blocks for h2 (ready after unit (2,1)) and h0 (ready
        # after (0,1)) — only h1's block is left for the tail.
        fill_g1 = [mk_qk(wq_sb, cs, ev_q) for cs in qcs]
        for h in range(HPC):
            for half in range(2):
                fill_g1.append(lambda h=h, half=half: trans_block(0, h, half))
        for qt in range(8):
            fill_g1.append(lambda qt=qt: outproj_chunk(qt, 0, psF, "f"))
            fill_g1.append(lambda qt=qt: outproj_chunk(qt, 1, psF, "f"))
        for half in range(2):
            fill_g1.append(lambda half=half: trans_block(1, 2, half))
        for half in range(2):
            fill_g1.append(lambda half=half: trans_block(1, 0, half))

        fillers = {0: fill_g0, 1: fill_g1}

        # ---- main pipelined emission ----
        sc_step(*seq[0])
        proj_qk(psF, wk_sb, slice(128, 256), ev_k)
        sc_step(*seq[1])
        proj_qk(psF, wk_sb, slice(256, 384), ev_k)
        proj_v(0)
        proj_v(1)
        for i, (h, g, j) in enumerate(seq):
            et = exp_step(h, g, j)
            ctx_step(h, g, j, et)
            if i + 2 < len(seq):
                sc_step(*seq[i + 2])
            if j == ST - 1:
                norm_evict(h, g, 0)
                norm_evict(h, g, 1)
            elif j < ST - 2:
                # no filler in the last two steps of a unit: keeps the DVE
                # queue clear so norm_evict frees the psB slots in time for
                # the next unit's first ctx matmuls.
                fq = fillers[g]
                if fq:
                    fq.pop(0)()

        # drain any unemitted fillers
        for fq in (fill_g0, fill_g1):
            while fq:
                fq.pop(0)()

        # ---- tail: transpose the h1 block of g1 + out-proj qt 8..15 ----
        trans_block(1, 1, 0)
        trans_block(1, 1, 1)
        for qt in range(8, 16):
            outproj_tail(qt, qt % 2)

        if debug:
            nc.sync.dma_start(out=d_qTA[:, :], in_=qTA.bitcast(F32))
            nc.sync.dma_start(out=d_kTA[:, :], in_=kTA.bitcast(F32))
            nc.sync.dma_start(out=d_qTB[:, :], in_=qTB.bitcast(F32))
            nc.sync.dma_start(out=d_kTB[:, :], in_=kTB.bitcast(F32))
            vf = work.tile([128, ST * HPC * (DH + 1)], F32, tag="dv", name="vf")
            nc.vector.tensor_copy(vf, v_sb.rearrange("p a b c -> p (a b c)"))
            nc.sync.dma_start(out=d_v[:, :], in_=vf)
            cf = work.tile([128, ST * DH3], F32, tag="dc", name="cf")
            nc.vector.tensor_copy(cf, ctx_sb.rearrange("p a b -> p (a b)"))
            nc.sync.dma_start(out=d_ctx[:, :], in_=cf)
            caf = work.tile([128, S], F32, tag="dca", name="caf")
            nc.vector.tensor_copy(caf, ctxTA)
            nc.sync.dma_start(out=d_ctxTA[:, :], in_=caf)
            cbf = work.tile([64, S], F32, tag="dcb", name="cbf")
            nc.vector.tensor_copy(cbf, ctxTB)
            nc.sync.dma_start(out=d_ctxTB[:, :], in_=cbf)

    nc.compile()
    return nc


def _w_rearrange(w):
    """[768, M] -> [128, 6*M] bf16: row p holds w[kt*128+p, :] for kt=0..5,
    so the device DMA is one contiguous segment per partition."""
    import ml_dtypes

    d, m = w.shape
    kt = d // 128
    return np.ascontiguousarray(
        w.reshape(kt, 128, m).transpose(1, 0, 2).reshape(128, kt * m)
    ).astype(ml_dtypes.bfloat16)


def _bias_block(bq, bk, col):
    # [128, 4]: col0 = bk heads01, col1 = bk head2 (rows 0:64),
    # col2 = bq heads01 (pre-scaled), col3 = bq head2 at rows 64:128
    blk = np.zeros((128, 4), np.float32)
    blk[:, 0] = bk[col : col + 128]
    blk[0:64, 1] = bk[col + 128 : col + 192]
    blk[:, 2] = bq[col : col + 128] * np.float32(0.125)
    blk[64:128, 3] = bq[col + 128 : col + 192] * np.float32(0.125)
    return blk


def _prep_in_maps(inputs):
    import ml_dtypes

    bf16 = ml_dtypes.bfloat16
    x = np.asarray(inputs["x"], dtype=np.float32)
    Wq = np.asarray(inputs["Wq"], dtype=np.float32)
    Wk = np.asarray(inputs["Wk"], dtype=np.float32)
    Wv = np.asarray(inputs["Wv"], dtype=np.float32)
    Wo = np.asarray(inputs["Wo"], dtype=np.float32)
    bq = np.asarray(inputs["bq"], dtype=np.float32)
    bk = np.asarray(inputs["bk"], dtype=np.float32)
    ident = np.eye(128, dtype=np.float32).astype(bf16)

    in_maps = []
    for c in range(NCORES):
        b = c // 4
        col = (c % 4) * DH3
        sl = slice(col, col + DH3)
        in_maps.append(
            {
                "xT": np.ascontiguousarray(x[b].T).astype(bf16),
                "wq": _w_rearrange(Wq[:, col : col + 128] * np.float32(0.125)),
                "wk": _w_rearrange(Wk[:, col : col + 128]),
                "wkq2": _w_rearrange(np.concatenate(
                    [
                        Wk[:, col + 128 : col + 192],
                        Wq[:, col + 128 : col + 192] * np.float32(0.125),
                    ],
                    axis=1,
                )),
                "wv": _w_rearrange(Wv[:, sl]),
                "wo": np.ascontiguousarray(Wo[sl, :]).astype(bf16),
                "bias": _bias_block(bq, bk, col),
                "ident": ident,
            }
        )
    return in_maps


def _combine(results, inputs):
    Wo = np.asarray(inputs["Wo"], dtype=np.float32)
    bv = np.asarray(inputs["bv"], dtype=np.float32)
    bo = np.asarray(inputs["bo"], dtype=np.float32)
    base = bv @ Wo + bo  # [D]
    out = np.empty((B, S, D), dtype=np.float32)
    for b in range(B):
        acc = results[4 * b]["out"].astype(np.float32)
        for c in range(4 * b + 1, 4 * b + 4):
            acc = acc + results[c]["out"].astype(np.float32)
        out[b] = acc + base
    return out


def run(inputs, trace: bool = False):
    """Run the 8-core kernel; returns (output, BassKernelResults)."""
    global _CACHED_NC
    if _CACHED_NC is None:
        _CACHED_NC = _build_nc()
    in_maps = _prep_in_maps(inputs)
    try:
        res = run_bass_kernel_spmd(
            _CACHED_NC, in_maps, core_ids=list(range(NCORES)), trace=trace
        )
    except ModuleNotFoundError:
        import os

        os.environ["BASS_NEVER_TRACE"] = "1"
        res = run_bass_kernel_spmd(
            _CACHED_NC, in_maps, core_ids=list(range(NCORES)), trace=False
        )
    return _combine(res.results, inputs), res


def kernel(**inputs) -> np.ndarray:
    out, _ = run(inputs)
    return out
